# revision 4
# baseline (speedup 1.0000x reference)
"""MoE experts kernel for Trainium2 (8 NeuronCores, expert-parallel).

Reference computation (per token t, top-k expert e with gate p):
    y[t] = sum_k p[t,k] * down_e @ (silu(x[t] @ gate_e) * (x[t] @ up_e))
with per-expert capacity CAP=1024 (tokens beyond capacity dropped).

Strategy:
  - Host: sort token assignments by expert (stable, matching jnp.argsort),
    build per-expert dense token buffers transposed to [H, NPAD] so the
    device kernel needs no transposes anywhere.
  - Device (SPMD over 8 cores, 8 experts/core): grouped GEMMs in float32r
    (full-rate PE fp32 mode, ~1e-4 rel err):
       G^T = gate^T-slices @ X^T   (accumulate over H chunks)
       U^T = up^T-slices   @ X^T
       Hm^T = silu(G^T) * U^T
       O^T  = down^T-slices @ Hm^T (accumulate over I chunks)
  - Host: gather rows back, apply routing weights, sum over top-k.
"""

import os
import sys

sys.path.insert(0, "/opt/trn_rl_repo")

import numpy as np

E, H, I, T, K = 64, 2048, 768, 4096, 8
CAP = 1024
NCORES = 8
EPC = E // NCORES  # experts per core
NH = H // 128  # 16 contraction chunks for gate/up
NI = I // 128  # 6 contraction chunks for down

_prog_cache = {}
LAST_EXEC_NS = None
LAST_RESULTS = None


def _groups(npad):
    ng = -(-npad // 512)
    w = -(-npad // ng)
    out = []
    s = 0
    while s < npad:
        e = min(s + w, npad)
        out.append((s, e))
        s = e
    return out


def _build_program(npad):
    import concourse.bacc as bacc
    import concourse.mybir as mybir
    from concourse.tile import TileContext

    f32 = mybir.dt.float32
    f32r = mybir.dt.float32r
    SILU = mybir.ActivationFunctionType.Silu

    groups = _groups(npad)

    nc = bacc.Bacc(None, target_bir_lowering=False)
    xT = nc.declare_dram_parameter("xT", [EPC, NH, 128, npad], f32r, isOutput=False)
    gw = nc.declare_dram_parameter("gw", [EPC, NI, 128, NH, 128], f32r, isOutput=False)
    uw = nc.declare_dram_parameter("uw", [EPC, NI, 128, NH, 128], f32r, isOutput=False)
    dw = nc.declare_dram_parameter("dw", [EPC, NH, 128, NI, 128], f32r, isOutput=False)
    yT = nc.declare_dram_parameter("yT", [EPC, NH, 128, npad], f32, isOutput=True)

    # SBUF budget (bytes/partition, 192KB cap): xt NH*bufs*npad*4, hm NI*2*npad*4,
    # weights ~38KB, out ~3*npad*4.  Drop xt to single-buffered for big npad.
    xt_bufs = 2 if npad <= 640 else 1

    with TileContext(nc) as tc:
        with (
            tc.sbuf_pool(name="xp", bufs=xt_bufs) as xp,
            tc.sbuf_pool(name="wp", bufs=2) as wp,
            tc.sbuf_pool(name="hp", bufs=2) as hp,
            tc.sbuf_pool(name="op", bufs=3) as op,
            tc.sbuf_pool(name="tp", bufs=3) as tp,
            tc.psum_pool(name="pp", bufs=2) as pp,
        ):
            for e in range(EPC):
                xts = []
                for h in range(NH):
                    t = xp.tile([128, npad], f32r, name=f"xt{h}", tag=f"xt{h}")
                    nc.sync.dma_start(out=t, in_=xT[e, h, :, :])
                    xts.append(t)
                hms = [
                    hp.tile([128, npad], f32r, name=f"hm{i}", tag=f"hm{i}")
                    for i in range(NI)
                ]
                for i in range(NI):
                    g_w = wp.tile([128, NH, 128], f32r, name="g_w", tag="g_w")
                    u_w = wp.tile([128, NH, 128], f32r, name="u_w", tag="u_w")
                    nc.sync.dma_start(out=g_w, in_=gw[e, i, :, :, :])
                    nc.sync.dma_start(out=u_w, in_=uw[e, i, :, :, :])
                    for g0, g1 in groups:
                        wdt = g1 - g0
                        psg = pp.tile([128, wdt], f32, name="psg", tag="psg")
                        psu = pp.tile([128, wdt], f32, name="psu", tag="psu")
                        for h in range(NH):
                            nc.tensor.matmul(
                                psg,
                                g_w[:, h, :],
                                xts[h][:, g0:g1],
                                start=(h == 0),
                                stop=(h == NH - 1),
                            )
                        for h in range(NH):
                            nc.tensor.matmul(
                                psu,
                                u_w[:, h, :],
                                xts[h][:, g0:g1],
                                start=(h == 0),
                                stop=(h == NH - 1),
                            )
                        sil = tp.tile([128, wdt], f32, name="sil", tag="sil")
                        nc.scalar.activation(sil, psg, SILU)
                        nc.vector.tensor_mul(hms[i][:, g0:g1], sil, psu)
                for h in range(NH):
                    d_w = wp.tile([128, NI, 128], f32r, name="d_w", tag="d_w")
                    nc.sync.dma_start(out=d_w, in_=dw[e, h, :, :, :])
                    ot = op.tile([128, npad], f32, name="ot", tag="ot")
                    for g0, g1 in groups:
                        wdt = g1 - g0
                        pso = pp.tile([128, wdt], f32, name="pso", tag="pso")
                        for i in range(NI):
                            nc.tensor.matmul(
                                pso,
                                d_w[:, i, :],
                                hms[i][:, g0:g1],
                                start=(i == 0),
                                stop=(i == NI - 1),
                            )
                        nc.scalar.activation(
                            ot[:, g0:g1], pso, mybir.ActivationFunctionType.Copy
                        )
                    nc.sync.dma_start(out=yT[e, h, :, :], in_=ot)
    nc.compile()
    return nc


def _install_ntff_hook_shim():
    """Provide antenv.axon_hooks (absent in this container) so that
    run_bass_kernel_spmd(trace=True) can capture NTFF profiles via the
    axon .so — mirrors trn_agent_boot.trn_boot's ctypes hook."""
    import types
    import ctypes
    import contextlib

    if "antenv.axon_hooks" in sys.modules:
        return
    so_path = "/opt/axon/libaxon_pjrt.so"
    lib = ctypes.CDLL(so_path)
    if not hasattr(lib, "axon_start_nrt_profile"):
        return
    lib.axon_start_nrt_profile.argtypes = [
        ctypes.POINTER(ctypes.c_int64),
        ctypes.c_size_t,
    ]
    lib.axon_start_nrt_profile.restype = ctypes.c_int64
    lib.axon_stop_nrt_profile.argtypes = [ctypes.c_char_p]
    lib.axon_stop_nrt_profile.restype = ctypes.c_int64

    @contextlib.contextmanager
    def _hook(output_dir, device_ids):
        import jax

        jax.devices()
        if device_ids:
            ids = (ctypes.c_int64 * len(device_ids))(*device_ids)
            rc = lib.axon_start_nrt_profile(ids, len(device_ids))
        else:
            rc = lib.axon_start_nrt_profile(None, 0)
        if rc != 0:
            raise RuntimeError(f"axon_start_nrt_profile rc={rc}")
        try:
            yield
        finally:
            n = lib.axon_stop_nrt_profile(str(output_dir).encode())
            print(f"profile: {n} file(s) written to {output_dir}", file=sys.stderr)

    mod = types.ModuleType("antenv.axon_hooks")
    mod.get_axon_ntff_profile_hook = lambda: _hook
    mod.set_axon_ntff_profile_hook = lambda h: None
    sys.modules["antenv.axon_hooks"] = mod


def kernel(
    hidden_states,
    routing_weights,
    selected_experts,
    gate_proj,
    up_proj,
    down_proj,
):
    global LAST_EXEC_NS, LAST_RESULTS
    from concourse.bass_utils import run_bass_kernel_spmd

    x = np.ascontiguousarray(np.asarray(hidden_states, dtype=np.float32))
    rw = np.asarray(routing_weights, dtype=np.float32)
    sel = np.asarray(selected_experts).astype(np.int64)
    gate = np.asarray(gate_proj, dtype=np.float32)
    up = np.asarray(up_proj, dtype=np.float32)
    down = np.asarray(down_proj, dtype=np.float32)

    # ---- host dispatch (mirrors reference's stable sort-by-expert) ----
    flat_e = sel.reshape(-1)
    order = np.argsort(flat_e, kind="stable")
    sorted_e = flat_e[order]
    counts = np.bincount(flat_e, minlength=E)
    offsets = np.concatenate([[0], np.cumsum(counts)[:-1]])
    pos = np.arange(flat_e.shape[0], dtype=np.int64) - offsets[sorted_e]

    npad = int(min(CAP, max(256, -(-counts.max() // 128) * 128)))
    keep = pos < npad  # npad >= min(max_count, CAP); beyond-CAP tokens dropped

    tok = order // K
    ke = sorted_e[keep]
    kp = pos[keep]

    # Dense per-expert buffers, transposed: xbufT[e] = X_e^T  [H, npad]
    xbufT = np.zeros((E, H, npad), dtype=np.float32)
    xbufT[ke, :, kp] = x[tok[keep]]

    # ---- weight/token layouts (contiguous per-DMA blocks) ----
    # gate/up slice for (e, i): [128p, NH, 128c] where [p, h, c] = W[h*128+p, i*128+c]
    gate_r = gate.reshape(E, NH, 128, NI, 128).transpose(0, 3, 2, 1, 4)
    up_r = up.reshape(E, NH, 128, NI, 128).transpose(0, 3, 2, 1, 4)
    # down slice for (e, h): [128p, NI, 128m] where [p, i, m] = W[i*128+p, h*128+m]
    down_r = down.reshape(E, NI, 128, NH, 128).transpose(0, 3, 2, 1, 4)
    # xT for (e, h): [128p, npad] = X_e^T rows h*128..h*128+127
    xT_r = xbufT.reshape(E, NH, 128, npad)

    nc = _prog_cache.get(npad)
    if nc is None:
        nc = _build_program(npad)
        _prog_cache[npad] = nc

    in_maps = []
    for c in range(NCORES):
        s = slice(c * EPC, (c + 1) * EPC)
        in_maps.append(
            {
                "xT": np.ascontiguousarray(xT_r[s]),
                "gw": np.ascontiguousarray(gate_r[s]),
                "uw": np.ascontiguousarray(up_r[s]),
                "dw": np.ascontiguousarray(down_r[s]),
            }
        )

    trace = bool(os.environ.get("BASS_MOE_TRACE"))
    kwargs = {}
    if trace:
        _install_ntff_hook_shim()
        tcores = os.environ.get("BASS_MOE_TRACE_CORES", "0")
        kwargs = dict(trace=True, trace_cores=[int(c) for c in tcores.split(",")])
    res = run_bass_kernel_spmd(nc, in_maps, core_ids=list(range(NCORES)), **kwargs)
    LAST_EXEC_NS = res.exec_time_ns
    LAST_RESULTS = res

    # ---- host combine ----
    # yT[c] : [EPC, NH, 128, npad] -> O_e [npad, H]
    o_all = np.concatenate([r["yT"] for r in res.results], axis=0)  # [E,NH,128,npad]
    o_all = o_all.reshape(E, H, npad)

    gathered = np.zeros((flat_e.shape[0], H), dtype=np.float32)
    gathered[order[keep]] = o_all[ke, :, kp]
    y = (gathered.reshape(T, K, H) * rw[:, :, None]).sum(axis=1, dtype=np.float32)
    return y.astype(np.float32)


# revision 6
# speedup vs baseline: 1.0165x; 1.0165x over previous
"""MoE experts kernel for Trainium2 (8 NeuronCores, expert-parallel).

Reference computation (per token t, top-k expert e with gate p):
    y[t] = sum_k p[t,k] * down_e @ (silu(x[t] @ gate_e) * (x[t] @ up_e))
with per-expert capacity CAP=1024 (tokens beyond capacity dropped).

Strategy:
  - Host: sort token assignments by expert (stable, matching jnp.argsort),
    build per-expert dense token buffers transposed to [H, NPAD] so the
    device kernel needs no transposes anywhere.
  - Device (SPMD over 8 cores, 8 experts/core): grouped GEMMs in float32r
    (full-rate PE fp32 mode, ~1e-4 rel err):
       G^T = gate^T-slices @ X^T   (accumulate over H chunks)
       U^T = up^T-slices   @ X^T
       Hm^T = silu(G^T) * U^T
       O^T  = down^T-slices @ Hm^T (accumulate over I chunks)
  - Host: gather rows back, apply routing weights, sum over top-k.
"""

import os
import sys

sys.path.insert(0, "/opt/trn_rl_repo")

import numpy as np

E, H, I, T, K = 64, 2048, 768, 4096, 8
CAP = 1024
NCORES = 8
EPC = E // NCORES  # experts per core
NH = H // 128  # 16 contraction chunks for gate/up
NI = I // 128  # 6 contraction chunks for down

_prog_cache = {}
LAST_EXEC_NS = None
LAST_RESULTS = None


def _groups(npad):
    ng = -(-npad // 512)
    w = -(-npad // ng)
    out = []
    s = 0
    while s < npad:
        e = min(s + w, npad)
        out.append((s, e))
        s = e
    return out


def _build_program(npad):
    import concourse.bacc as bacc
    import concourse.mybir as mybir
    from concourse.tile import TileContext

    f32 = mybir.dt.float32
    f32r = mybir.dt.float32r
    SILU = mybir.ActivationFunctionType.Silu

    groups = _groups(npad)

    nc = bacc.Bacc(None, target_bir_lowering=False)
    xT = nc.declare_dram_parameter("xT", [EPC, NH, 128, npad], f32r, isOutput=False)
    gw = nc.declare_dram_parameter("gw", [EPC, NI, 128, NH, 128], f32r, isOutput=False)
    uw = nc.declare_dram_parameter("uw", [EPC, NI, 128, NH, 128], f32r, isOutput=False)
    dw = nc.declare_dram_parameter("dw", [EPC, NH, 128, NI, 128], f32r, isOutput=False)
    yT = nc.declare_dram_parameter("yT", [EPC, NH, 128, npad], f32, isOutput=True)

    # SBUF budget (bytes/partition, 192KB cap): xt NH*bufs*npad*4, hm NI*2*npad*4,
    # weights ~38KB, out ~3*npad*4.  Drop xt to single-buffered for big npad.
    xt_bufs = 2 if npad <= 640 else 1

    with TileContext(nc) as tc:
        with (
            tc.sbuf_pool(name="xp", bufs=xt_bufs) as xp,
            tc.sbuf_pool(name="wp", bufs=2) as wp,
            tc.sbuf_pool(name="hp", bufs=2) as hp,
            tc.sbuf_pool(name="op", bufs=3) as op,
            tc.sbuf_pool(name="tp", bufs=3) as tp,
            tc.psum_pool(name="pp", bufs=2) as pp,
        ):
            for e in range(EPC):
                xts = []
                for h in range(NH):
                    t = xp.tile([128, npad], f32r, name=f"xt{h}", tag=f"xt{h}")
                    nc.sync.dma_start(out=t, in_=xT[e, h, :, :])
                    xts.append(t)
                hms = [
                    hp.tile([128, npad], f32r, name=f"hm{i}", tag=f"hm{i}")
                    for i in range(NI)
                ]
                for i in range(NI):
                    g_w = wp.tile([128, NH, 128], f32r, name="g_w", tag="g_w")
                    u_w = wp.tile([128, NH, 128], f32r, name="u_w", tag="u_w")
                    nc.sync.dma_start(out=g_w, in_=gw[e, i, :, :, :])
                    nc.sync.dma_start(out=u_w, in_=uw[e, i, :, :, :])
                    for g0, g1 in groups:
                        wdt = g1 - g0
                        psg = pp.tile([128, wdt], f32, name="psg", tag="psg")
                        psu = pp.tile([128, wdt], f32, name="psu", tag="psu")
                        for h in range(NH):
                            nc.tensor.matmul(
                                psg,
                                g_w[:, h, :],
                                xts[h][:, g0:g1],
                                start=(h == 0),
                                stop=(h == NH - 1),
                            )
                        for h in range(NH):
                            nc.tensor.matmul(
                                psu,
                                u_w[:, h, :],
                                xts[h][:, g0:g1],
                                start=(h == 0),
                                stop=(h == NH - 1),
                            )
                        sil = tp.tile([128, wdt], f32, name="sil", tag="sil")
                        nc.scalar.activation(sil, psg, SILU)
                        nc.vector.tensor_mul(hms[i][:, g0:g1], sil, psu)
                for h in range(NH):
                    d_w = wp.tile([128, NI, 128], f32r, name="d_w", tag="d_w")
                    nc.sync.dma_start(out=d_w, in_=dw[e, h, :, :, :])
                    ot = op.tile([128, npad], f32, name="ot", tag="ot")
                    for g0, g1 in groups:
                        wdt = g1 - g0
                        pso = pp.tile([128, wdt], f32, name="pso", tag="pso")
                        for i in range(NI):
                            nc.tensor.matmul(
                                pso,
                                d_w[:, i, :],
                                hms[i][:, g0:g1],
                                start=(i == 0),
                                stop=(i == NI - 1),
                            )
                        nc.scalar.activation(
                            ot[:, g0:g1], pso, mybir.ActivationFunctionType.Copy
                        )
                    nc.sync.dma_start(out=yT[e, h, :, :], in_=ot)
    nc.compile()
    return nc


def _install_neff_cache():
    """Cache walrus NEFF compiles on disk keyed by BIR hash (compile of the
    ~11k-instruction program takes minutes; the BIR is deterministic)."""
    import hashlib
    import shutil

    import concourse.bass2jax as bass2jax
    from concourse.bass_utils import compile_bir_kernel as _orig

    if getattr(bass2jax.compile_bir_kernel, "_moe_cached", False):
        return
    cache_dir = os.environ.get("BASS_NEFF_CACHE", "/tmp/bass_neff_cache")
    os.makedirs(cache_dir, exist_ok=True)

    def cached(bir_json, tmpdir, neff_name="file.neff"):
        key = hashlib.sha256(bir_json).hexdigest()[:24]
        cpath = os.path.join(cache_dir, key + ".neff")
        dst = os.path.join(tmpdir, neff_name)
        if os.path.exists(cpath):
            shutil.copy(cpath, dst)
            return dst
        out = _orig(bir_json, tmpdir, neff_name)
        try:
            shutil.copy(out, cpath)
        except OSError:
            pass
        return out

    cached._moe_cached = True
    bass2jax.compile_bir_kernel = cached


def _install_ntff_hook_shim():
    """Provide antenv.axon_hooks (absent in this container) so that
    run_bass_kernel_spmd(trace=True) can capture NTFF profiles via the
    axon .so — mirrors trn_agent_boot.trn_boot's ctypes hook."""
    import types
    import ctypes
    import contextlib

    if "antenv.axon_hooks" in sys.modules:
        return
    so_path = "/opt/axon/libaxon_pjrt.so"
    lib = ctypes.CDLL(so_path)
    if not hasattr(lib, "axon_start_nrt_profile"):
        return
    lib.axon_start_nrt_profile.argtypes = [
        ctypes.POINTER(ctypes.c_int64),
        ctypes.c_size_t,
    ]
    lib.axon_start_nrt_profile.restype = ctypes.c_int64
    lib.axon_stop_nrt_profile.argtypes = [ctypes.c_char_p]
    lib.axon_stop_nrt_profile.restype = ctypes.c_int64

    @contextlib.contextmanager
    def _hook(output_dir, device_ids):
        import jax

        jax.devices()
        if device_ids:
            ids = (ctypes.c_int64 * len(device_ids))(*device_ids)
            rc = lib.axon_start_nrt_profile(ids, len(device_ids))
        else:
            rc = lib.axon_start_nrt_profile(None, 0)
        if rc != 0:
            raise RuntimeError(f"axon_start_nrt_profile rc={rc}")
        try:
            yield
        finally:
            n = lib.axon_stop_nrt_profile(str(output_dir).encode())
            print(f"profile: {n} file(s) written to {output_dir}", file=sys.stderr)

    mod = types.ModuleType("antenv.axon_hooks")
    mod.get_axon_ntff_profile_hook = lambda: _hook
    mod.set_axon_ntff_profile_hook = lambda h: None
    sys.modules["antenv.axon_hooks"] = mod


def kernel(
    hidden_states,
    routing_weights,
    selected_experts,
    gate_proj,
    up_proj,
    down_proj,
):
    global LAST_EXEC_NS, LAST_RESULTS
    from concourse.bass_utils import run_bass_kernel_spmd

    _install_neff_cache()

    x = np.ascontiguousarray(np.asarray(hidden_states, dtype=np.float32))
    rw = np.asarray(routing_weights, dtype=np.float32)
    sel = np.asarray(selected_experts).astype(np.int64)
    gate = np.asarray(gate_proj, dtype=np.float32)
    up = np.asarray(up_proj, dtype=np.float32)
    down = np.asarray(down_proj, dtype=np.float32)

    # ---- host dispatch (mirrors reference's stable sort-by-expert) ----
    flat_e = sel.reshape(-1)
    order = np.argsort(flat_e, kind="stable")
    sorted_e = flat_e[order]
    counts = np.bincount(flat_e, minlength=E)
    offsets = np.concatenate([[0], np.cumsum(counts)[:-1]])
    pos = np.arange(flat_e.shape[0], dtype=np.int64) - offsets[sorted_e]

    npad = int(min(CAP, max(256, -(-counts.max() // 128) * 128)))
    keep = pos < npad  # npad >= min(max_count, CAP); beyond-CAP tokens dropped

    tok = order // K
    ke = sorted_e[keep]
    kp = pos[keep]

    # Dense per-expert buffers, transposed: xbufT[e] = X_e^T  [H, npad]
    xbufT = np.zeros((E, H, npad), dtype=np.float32)
    xbufT[ke, :, kp] = x[tok[keep]]

    # ---- weight/token layouts (contiguous per-DMA blocks) ----
    # gate/up slice for (e, i): [128p, NH, 128c] where [p, h, c] = W[h*128+p, i*128+c]
    gate_r = gate.reshape(E, NH, 128, NI, 128).transpose(0, 3, 2, 1, 4)
    up_r = up.reshape(E, NH, 128, NI, 128).transpose(0, 3, 2, 1, 4)
    # down slice for (e, h): [128p, NI, 128m] where [p, i, m] = W[i*128+p, h*128+m]
    down_r = down.reshape(E, NI, 128, NH, 128).transpose(0, 3, 2, 1, 4)
    # xT for (e, h): [128p, npad] = X_e^T rows h*128..h*128+127
    xT_r = xbufT.reshape(E, NH, 128, npad)

    nc = _prog_cache.get(npad)
    if nc is None:
        nc = _build_program(npad)
        _prog_cache[npad] = nc

    in_maps = []
    for c in range(NCORES):
        s = slice(c * EPC, (c + 1) * EPC)
        in_maps.append(
            {
                "xT": np.ascontiguousarray(xT_r[s]),
                "gw": np.ascontiguousarray(gate_r[s]),
                "uw": np.ascontiguousarray(up_r[s]),
                "dw": np.ascontiguousarray(down_r[s]),
            }
        )

    trace = bool(os.environ.get("BASS_MOE_TRACE"))
    kwargs = {}
    if trace:
        _install_ntff_hook_shim()
        tcores = os.environ.get("BASS_MOE_TRACE_CORES", "0")
        kwargs = dict(trace=True, trace_cores=[int(c) for c in tcores.split(",")])
    res = run_bass_kernel_spmd(nc, in_maps, core_ids=list(range(NCORES)), **kwargs)
    LAST_EXEC_NS = res.exec_time_ns
    LAST_RESULTS = res

    # ---- host combine ----
    # yT[c] : [EPC, NH, 128, npad] -> O_e [npad, H]
    o_all = np.concatenate([r["yT"] for r in res.results], axis=0)  # [E,NH,128,npad]
    o_all = o_all.reshape(E, H, npad)

    gathered = np.zeros((flat_e.shape[0], H), dtype=np.float32)
    gathered[order[keep]] = o_all[ke, :, kp]
    y = (gathered.reshape(T, K, H) * rw[:, :, None]).sum(axis=1, dtype=np.float32)
    return y.astype(np.float32)


# revision 8
# speedup vs baseline: 1.2757x; 1.2550x over previous
"""MoE experts kernel for Trainium2 (8 NeuronCores, expert-parallel).

Reference computation (per token t, top-k expert e with gate p):
    y[t] = sum_k p[t,k] * down_e @ (silu(x[t] @ gate_e) * (x[t] @ up_e))
with per-expert capacity CAP=1024 (tokens beyond capacity dropped).

Strategy:
  - Host: sort token assignments by expert (stable, matching jnp.argsort),
    build per-expert dense token buffers transposed to [H, NPAD] so the
    device kernel needs no transposes anywhere.
  - Device (SPMD over 8 cores, 8 experts/core): grouped GEMMs in float32r
    (full-rate PE fp32 mode, ~1e-4 rel err):
       G^T = gate^T-slices @ X^T   (accumulate over H chunks)
       U^T = up^T-slices   @ X^T
       Hm^T = silu(G^T) * U^T
       O^T  = down^T-slices @ Hm^T (accumulate over I chunks)
  - Host: gather rows back, apply routing weights, sum over top-k.
"""

import os
import sys

sys.path.insert(0, "/opt/trn_rl_repo")

import numpy as np

E, H, I, T, K = 64, 2048, 768, 4096, 8
CAP = 1024
NCORES = 8
EPC = E // NCORES  # experts per core
NH = H // 128  # 16 contraction chunks for gate/up
NI = I // 128  # 6 contraction chunks for down

_prog_cache = {}
LAST_EXEC_NS = None
LAST_RESULTS = None


def _groups(npad):
    ng = -(-npad // 512)
    w = -(-npad // ng)
    out = []
    s = 0
    while s < npad:
        e = min(s + w, npad)
        out.append((s, e))
        s = e
    return out


MM_DT = "float16"  # matmul operand dtype: float16 | float32r


def _build_program(npad):
    import concourse.bacc as bacc
    import concourse.mybir as mybir
    from concourse.tile import TileContext

    f32 = mybir.dt.float32
    mdt = getattr(mybir.dt, MM_DT)
    SILU = mybir.ActivationFunctionType.Silu

    groups = _groups(npad)

    nc = bacc.Bacc(None, target_bir_lowering=False)
    xT = nc.declare_dram_parameter("xT", [EPC, NH, 128, npad], mdt, isOutput=False)
    gw = nc.declare_dram_parameter("gw", [EPC, NI, 128, NH, 128], mdt, isOutput=False)
    uw = nc.declare_dram_parameter("uw", [EPC, NI, 128, NH, 128], mdt, isOutput=False)
    dw = nc.declare_dram_parameter("dw", [EPC, NH, 128, NI, 128], mdt, isOutput=False)
    yT = nc.declare_dram_parameter("yT", [EPC, NH, 128, npad], f32, isOutput=True)

    xt_bufs = 2

    with TileContext(nc) as tc:
        with (
            tc.sbuf_pool(name="xp", bufs=xt_bufs) as xp,
            tc.sbuf_pool(name="wp", bufs=3) as wp,
            tc.sbuf_pool(name="hp", bufs=2) as hp,
            tc.sbuf_pool(name="op", bufs=3) as op,
            tc.sbuf_pool(name="tp", bufs=3) as tp,
            tc.psum_pool(name="pp", bufs=2) as pp,
        ):
            for e in range(EPC):
                xts = []
                for h in range(NH):
                    t = xp.tile([128, npad], mdt, name=f"xt{h}", tag=f"xt{h}")
                    nc.sync.dma_start(out=t, in_=xT[e, h, :, :])
                    xts.append(t)
                hms = [
                    hp.tile([128, npad], mdt, name=f"hm{i}", tag=f"hm{i}")
                    for i in range(NI)
                ]
                for i in range(NI):
                    g_w = wp.tile([128, NH, 128], mdt, name="g_w", tag="g_w")
                    u_w = wp.tile([128, NH, 128], mdt, name="u_w", tag="u_w")
                    nc.sync.dma_start(out=g_w, in_=gw[e, i, :, :, :])
                    nc.sync.dma_start(out=u_w, in_=uw[e, i, :, :, :])
                    for g0, g1 in groups:
                        wdt = g1 - g0
                        psg = pp.tile([128, wdt], f32, name="psg", tag="psg")
                        psu = pp.tile([128, wdt], f32, name="psu", tag="psu")
                        for h in range(NH):
                            nc.tensor.matmul(
                                psg,
                                g_w[:, h, :],
                                xts[h][:, g0:g1],
                                start=(h == 0),
                                stop=(h == NH - 1),
                            )
                        for h in range(NH):
                            nc.tensor.matmul(
                                psu,
                                u_w[:, h, :],
                                xts[h][:, g0:g1],
                                start=(h == 0),
                                stop=(h == NH - 1),
                            )
                        sil = tp.tile([128, wdt], f32, name="sil", tag="sil")
                        nc.scalar.activation(sil, psg, SILU)
                        nc.vector.tensor_mul(hms[i][:, g0:g1], sil, psu)
                for h in range(NH):
                    d_w = wp.tile([128, NI, 128], mdt, name="d_w", tag="d_w")
                    nc.sync.dma_start(out=d_w, in_=dw[e, h, :, :, :])
                    ot = op.tile([128, npad], f32, name="ot", tag="ot")
                    for gi, (g0, g1) in enumerate(groups):
                        wdt = g1 - g0
                        pso = pp.tile([128, wdt], f32, name="pso", tag="pso")
                        for i in range(NI):
                            nc.tensor.matmul(
                                pso,
                                d_w[:, i, :],
                                hms[i][:, g0:g1],
                                start=(i == 0),
                                stop=(i == NI - 1),
                            )
                        if (h + gi) % 2 == 0:
                            nc.vector.tensor_copy(ot[:, g0:g1], pso)
                        else:
                            nc.scalar.activation(
                                ot[:, g0:g1], pso, mybir.ActivationFunctionType.Copy
                            )
                    nc.sync.dma_start(out=yT[e, h, :, :], in_=ot)
    nc.compile()
    return nc


def _install_neff_cache():
    """Cache walrus NEFF compiles on disk keyed by BIR hash (compile of the
    ~11k-instruction program takes minutes; the BIR is deterministic)."""
    import hashlib
    import shutil

    import concourse.bass2jax as bass2jax
    from concourse.bass_utils import compile_bir_kernel as _orig

    if getattr(bass2jax.compile_bir_kernel, "_moe_cached", False):
        return
    cache_dir = os.environ.get("BASS_NEFF_CACHE", "/tmp/bass_neff_cache")
    os.makedirs(cache_dir, exist_ok=True)

    def cached(bir_json, tmpdir, neff_name="file.neff"):
        key = hashlib.sha256(bir_json).hexdigest()[:24]
        cpath = os.path.join(cache_dir, key + ".neff")
        dst = os.path.join(tmpdir, neff_name)
        if os.path.exists(cpath):
            shutil.copy(cpath, dst)
            return dst
        out = _orig(bir_json, tmpdir, neff_name)
        try:
            shutil.copy(out, cpath)
        except OSError:
            pass
        return out

    cached._moe_cached = True
    bass2jax.compile_bir_kernel = cached


def _install_ntff_hook_shim():
    """Provide antenv.axon_hooks (absent in this container) so that
    run_bass_kernel_spmd(trace=True) can capture NTFF profiles via the
    axon .so — mirrors trn_agent_boot.trn_boot's ctypes hook."""
    import types
    import ctypes
    import contextlib

    if "antenv.axon_hooks" in sys.modules:
        return
    so_path = "/opt/axon/libaxon_pjrt.so"
    lib = ctypes.CDLL(so_path)
    if not hasattr(lib, "axon_start_nrt_profile"):
        return
    lib.axon_start_nrt_profile.argtypes = [
        ctypes.POINTER(ctypes.c_int64),
        ctypes.c_size_t,
    ]
    lib.axon_start_nrt_profile.restype = ctypes.c_int64
    lib.axon_stop_nrt_profile.argtypes = [ctypes.c_char_p]
    lib.axon_stop_nrt_profile.restype = ctypes.c_int64

    @contextlib.contextmanager
    def _hook(output_dir, device_ids):
        import jax

        jax.devices()
        if device_ids:
            ids = (ctypes.c_int64 * len(device_ids))(*device_ids)
            rc = lib.axon_start_nrt_profile(ids, len(device_ids))
        else:
            rc = lib.axon_start_nrt_profile(None, 0)
        if rc != 0:
            raise RuntimeError(f"axon_start_nrt_profile rc={rc}")
        try:
            yield
        finally:
            n = lib.axon_stop_nrt_profile(str(output_dir).encode())
            print(f"profile: {n} file(s) written to {output_dir}", file=sys.stderr)

    mod = types.ModuleType("antenv.axon_hooks")
    mod.get_axon_ntff_profile_hook = lambda: _hook
    mod.set_axon_ntff_profile_hook = lambda h: None
    sys.modules["antenv.axon_hooks"] = mod


def kernel(
    hidden_states,
    routing_weights,
    selected_experts,
    gate_proj,
    up_proj,
    down_proj,
):
    global LAST_EXEC_NS, LAST_RESULTS
    from concourse.bass_utils import run_bass_kernel_spmd

    _install_neff_cache()

    x = np.ascontiguousarray(np.asarray(hidden_states, dtype=np.float32))
    rw = np.asarray(routing_weights, dtype=np.float32)
    sel = np.asarray(selected_experts).astype(np.int64)
    gate = np.asarray(gate_proj, dtype=np.float32)
    up = np.asarray(up_proj, dtype=np.float32)
    down = np.asarray(down_proj, dtype=np.float32)

    # ---- host dispatch (mirrors reference's stable sort-by-expert) ----
    flat_e = sel.reshape(-1)
    order = np.argsort(flat_e, kind="stable")
    sorted_e = flat_e[order]
    counts = np.bincount(flat_e, minlength=E)
    offsets = np.concatenate([[0], np.cumsum(counts)[:-1]])
    pos = np.arange(flat_e.shape[0], dtype=np.int64) - offsets[sorted_e]

    npad = int(min(CAP, max(256, -(-counts.max() // 128) * 128)))
    keep = pos < npad  # npad >= min(max_count, CAP); beyond-CAP tokens dropped

    tok = order // K
    ke = sorted_e[keep]
    kp = pos[keep]

    # Dense per-expert buffers, transposed: xbufT[e] = X_e^T  [H, npad]
    xbufT = np.zeros((E, H, npad), dtype=np.float32)
    xbufT[ke, :, kp] = x[tok[keep]]

    # ---- weight/token layouts (contiguous per-DMA blocks) ----
    # gate/up slice for (e, i): [128p, NH, 128c] where [p, h, c] = W[h*128+p, i*128+c]
    gate_r = gate.reshape(E, NH, 128, NI, 128).transpose(0, 3, 2, 1, 4)
    up_r = up.reshape(E, NH, 128, NI, 128).transpose(0, 3, 2, 1, 4)
    # down slice for (e, h): [128p, NI, 128m] where [p, i, m] = W[i*128+p, h*128+m]
    down_r = down.reshape(E, NI, 128, NH, 128).transpose(0, 3, 2, 1, 4)
    # xT for (e, h): [128p, npad] = X_e^T rows h*128..h*128+127
    xT_r = xbufT.reshape(E, NH, 128, npad)

    nc = _prog_cache.get(npad)
    if nc is None:
        nc = _build_program(npad)
        _prog_cache[npad] = nc

    mm_np = np.float16 if MM_DT == "float16" else np.float32
    in_maps = []
    for c in range(NCORES):
        s = slice(c * EPC, (c + 1) * EPC)
        in_maps.append(
            {
                "xT": np.ascontiguousarray(xT_r[s], dtype=mm_np),
                "gw": np.ascontiguousarray(gate_r[s], dtype=mm_np),
                "uw": np.ascontiguousarray(up_r[s], dtype=mm_np),
                "dw": np.ascontiguousarray(down_r[s], dtype=mm_np),
            }
        )

    trace = bool(os.environ.get("BASS_MOE_TRACE"))
    kwargs = {}
    if trace:
        _install_ntff_hook_shim()
        tcores = os.environ.get("BASS_MOE_TRACE_CORES", "0")
        kwargs = dict(trace=True, trace_cores=[int(c) for c in tcores.split(",")])
    res = run_bass_kernel_spmd(nc, in_maps, core_ids=list(range(NCORES)), **kwargs)
    LAST_EXEC_NS = res.exec_time_ns
    LAST_RESULTS = res

    # ---- host combine ----
    # yT[c] : [EPC, NH, 128, npad] -> O_e [npad, H]
    o_all = np.concatenate([r["yT"] for r in res.results], axis=0)  # [E,NH,128,npad]
    o_all = o_all.reshape(E, H, npad)

    gathered = np.zeros((flat_e.shape[0], H), dtype=np.float32)
    gathered[order[keep]] = o_all[ke, :, kp]
    y = (gathered.reshape(T, K, H) * rw[:, :, None]).sum(axis=1, dtype=np.float32)
    return y.astype(np.float32)


# revision 9
# speedup vs baseline: 1.3065x; 1.0241x over previous
"""MoE experts kernel for Trainium2 (8 NeuronCores, expert-parallel).

Reference computation (per token t, top-k expert e with gate p):
    y[t] = sum_k p[t,k] * down_e @ (silu(x[t] @ gate_e) * (x[t] @ up_e))
with per-expert capacity CAP=1024 (tokens beyond capacity dropped).

Strategy:
  - Host: sort token assignments by expert (stable, matching jnp.argsort),
    build per-expert dense token buffers transposed to [H, NPAD] so the
    device kernel needs no transposes anywhere.
  - Device (SPMD over 8 cores, 8 experts/core): grouped GEMMs in float32r
    (full-rate PE fp32 mode, ~1e-4 rel err):
       G^T = gate^T-slices @ X^T   (accumulate over H chunks)
       U^T = up^T-slices   @ X^T
       Hm^T = silu(G^T) * U^T
       O^T  = down^T-slices @ Hm^T (accumulate over I chunks)
  - Host: gather rows back, apply routing weights, sum over top-k.
"""

import os
import sys

sys.path.insert(0, "/opt/trn_rl_repo")

import numpy as np

E, H, I, T, K = 64, 2048, 768, 4096, 8
CAP = 1024
NCORES = 8
EPC = E // NCORES  # experts per core
NH = H // 128  # 16 contraction chunks for gate/up
NI = I // 128  # 6 contraction chunks for down

_prog_cache = {}
LAST_EXEC_NS = None
LAST_RESULTS = None


def _groups(npad):
    ng = -(-npad // 512)
    w = -(-npad // ng)
    out = []
    s = 0
    while s < npad:
        e = min(s + w, npad)
        out.append((s, e))
        s = e
    return out


MM_DT = "float16"  # matmul operand dtype: float16 | float32r


def _build_program(npad):
    import concourse.bacc as bacc
    import concourse.mybir as mybir
    from concourse.tile import TileContext

    f32 = mybir.dt.float32
    mdt = getattr(mybir.dt, MM_DT)
    SILU = mybir.ActivationFunctionType.Silu

    groups = _groups(npad)

    nc = bacc.Bacc(None, target_bir_lowering=False)
    xT = nc.declare_dram_parameter("xT", [EPC, NH, 128, npad], mdt, isOutput=False)
    gw = nc.declare_dram_parameter("gw", [EPC, NI, 128, NH, 128], mdt, isOutput=False)
    uw = nc.declare_dram_parameter("uw", [EPC, NI, 128, NH, 128], mdt, isOutput=False)
    dw = nc.declare_dram_parameter("dw", [EPC, NH, 128, NI, 128], mdt, isOutput=False)
    yT = nc.declare_dram_parameter("yT", [EPC, NH, 128, npad], f32, isOutput=True)

    xt_bufs = 2

    with TileContext(nc) as tc:
        with (
            tc.sbuf_pool(name="xp", bufs=xt_bufs) as xp,
            tc.sbuf_pool(name="wp", bufs=3) as wp,
            tc.sbuf_pool(name="hp", bufs=2) as hp,
            tc.sbuf_pool(name="op", bufs=3) as op,
            tc.sbuf_pool(name="tp", bufs=3) as tp,
            tc.psum_pool(name="pp", bufs=2) as pp,
        ):
            for e in range(EPC):
                xts = []
                for h in range(NH):
                    t = xp.tile([128, npad], mdt, name=f"xt{h}", tag=f"xt{h}")
                    nc.gpsimd.dma_start(out=t, in_=xT[e, h, :, :])
                    xts.append(t)
                hms = [
                    hp.tile([128, npad], mdt, name=f"hm{i}", tag=f"hm{i}")
                    for i in range(NI)
                ]
                for i in range(NI):
                    g_w = wp.tile([128, NH, 128], mdt, name="g_w", tag="g_w")
                    u_w = wp.tile([128, NH, 128], mdt, name="u_w", tag="u_w")
                    nc.sync.dma_start(out=g_w, in_=gw[e, i, :, :, :])
                    nc.sync.dma_start(out=u_w, in_=uw[e, i, :, :, :])
                    for g0, g1 in groups:
                        wdt = g1 - g0
                        psg = pp.tile([128, wdt], f32, name="psg", tag="psg")
                        psu = pp.tile([128, wdt], f32, name="psu", tag="psu")
                        for h in range(NH):
                            nc.tensor.matmul(
                                psg,
                                g_w[:, h, :],
                                xts[h][:, g0:g1],
                                start=(h == 0),
                                stop=(h == NH - 1),
                            )
                        for h in range(NH):
                            nc.tensor.matmul(
                                psu,
                                u_w[:, h, :],
                                xts[h][:, g0:g1],
                                start=(h == 0),
                                stop=(h == NH - 1),
                            )
                        sil = tp.tile([128, wdt], f32, name="sil", tag="sil")
                        nc.scalar.activation(sil, psg, SILU)
                        nc.vector.tensor_mul(hms[i][:, g0:g1], sil, psu)
                for h in range(NH):
                    d_w = wp.tile([128, NI, 128], mdt, name="d_w", tag="d_w")
                    nc.sync.dma_start(out=d_w, in_=dw[e, h, :, :, :])
                    ot = op.tile([128, npad], f32, name="ot", tag="ot")
                    for gi, (g0, g1) in enumerate(groups):
                        wdt = g1 - g0
                        pso = pp.tile([128, wdt], f32, name="pso", tag="pso")
                        for i in range(NI):
                            nc.tensor.matmul(
                                pso,
                                d_w[:, i, :],
                                hms[i][:, g0:g1],
                                start=(i == 0),
                                stop=(i == NI - 1),
                            )
                        if (h + gi) % 2 == 0:
                            nc.vector.tensor_copy(ot[:, g0:g1], pso)
                        else:
                            nc.scalar.activation(
                                ot[:, g0:g1], pso, mybir.ActivationFunctionType.Copy
                            )
                    nc.gpsimd.dma_start(out=yT[e, h, :, :], in_=ot)
    nc.compile()
    return nc


def _install_neff_cache():
    """Cache walrus NEFF compiles on disk keyed by BIR hash (compile of the
    ~11k-instruction program takes minutes; the BIR is deterministic)."""
    import hashlib
    import shutil

    import concourse.bass2jax as bass2jax
    from concourse.bass_utils import compile_bir_kernel as _orig

    if getattr(bass2jax.compile_bir_kernel, "_moe_cached", False):
        return
    cache_dir = os.environ.get("BASS_NEFF_CACHE", "/tmp/bass_neff_cache")
    os.makedirs(cache_dir, exist_ok=True)

    def cached(bir_json, tmpdir, neff_name="file.neff"):
        key = hashlib.sha256(bir_json).hexdigest()[:24]
        cpath = os.path.join(cache_dir, key + ".neff")
        dst = os.path.join(tmpdir, neff_name)
        if os.path.exists(cpath):
            shutil.copy(cpath, dst)
            return dst
        out = _orig(bir_json, tmpdir, neff_name)
        try:
            shutil.copy(out, cpath)
        except OSError:
            pass
        return out

    cached._moe_cached = True
    bass2jax.compile_bir_kernel = cached


def _install_ntff_hook_shim():
    """Provide antenv.axon_hooks (absent in this container) so that
    run_bass_kernel_spmd(trace=True) can capture NTFF profiles via the
    axon .so — mirrors trn_agent_boot.trn_boot's ctypes hook."""
    import types
    import ctypes
    import contextlib

    if "antenv.axon_hooks" in sys.modules:
        return
    so_path = "/opt/axon/libaxon_pjrt.so"
    lib = ctypes.CDLL(so_path)
    if not hasattr(lib, "axon_start_nrt_profile"):
        return
    lib.axon_start_nrt_profile.argtypes = [
        ctypes.POINTER(ctypes.c_int64),
        ctypes.c_size_t,
    ]
    lib.axon_start_nrt_profile.restype = ctypes.c_int64
    lib.axon_stop_nrt_profile.argtypes = [ctypes.c_char_p]
    lib.axon_stop_nrt_profile.restype = ctypes.c_int64

    @contextlib.contextmanager
    def _hook(output_dir, device_ids):
        import jax

        jax.devices()
        if device_ids:
            ids = (ctypes.c_int64 * len(device_ids))(*device_ids)
            rc = lib.axon_start_nrt_profile(ids, len(device_ids))
        else:
            rc = lib.axon_start_nrt_profile(None, 0)
        if rc != 0:
            raise RuntimeError(f"axon_start_nrt_profile rc={rc}")
        try:
            yield
        finally:
            n = lib.axon_stop_nrt_profile(str(output_dir).encode())
            print(f"profile: {n} file(s) written to {output_dir}", file=sys.stderr)

    mod = types.ModuleType("antenv.axon_hooks")
    mod.get_axon_ntff_profile_hook = lambda: _hook
    mod.set_axon_ntff_profile_hook = lambda h: None
    sys.modules["antenv.axon_hooks"] = mod


def kernel(
    hidden_states,
    routing_weights,
    selected_experts,
    gate_proj,
    up_proj,
    down_proj,
):
    global LAST_EXEC_NS, LAST_RESULTS
    from concourse.bass_utils import run_bass_kernel_spmd

    _install_neff_cache()

    x = np.ascontiguousarray(np.asarray(hidden_states, dtype=np.float32))
    rw = np.asarray(routing_weights, dtype=np.float32)
    sel = np.asarray(selected_experts).astype(np.int64)
    gate = np.asarray(gate_proj, dtype=np.float32)
    up = np.asarray(up_proj, dtype=np.float32)
    down = np.asarray(down_proj, dtype=np.float32)

    # ---- host dispatch (mirrors reference's stable sort-by-expert) ----
    flat_e = sel.reshape(-1)
    order = np.argsort(flat_e, kind="stable")
    sorted_e = flat_e[order]
    counts = np.bincount(flat_e, minlength=E)
    offsets = np.concatenate([[0], np.cumsum(counts)[:-1]])
    pos = np.arange(flat_e.shape[0], dtype=np.int64) - offsets[sorted_e]

    npad = int(min(CAP, max(256, -(-int(counts.max()) // 64) * 64)))
    keep = pos < npad  # npad >= min(max_count, CAP); beyond-CAP tokens dropped

    tok = order // K
    ke = sorted_e[keep]
    kp = pos[keep]

    # Dense per-expert buffers, transposed: xbufT[e] = X_e^T  [H, npad]
    xbufT = np.zeros((E, H, npad), dtype=np.float32)
    xbufT[ke, :, kp] = x[tok[keep]]

    # ---- weight/token layouts (contiguous per-DMA blocks) ----
    # gate/up slice for (e, i): [128p, NH, 128c] where [p, h, c] = W[h*128+p, i*128+c]
    gate_r = gate.reshape(E, NH, 128, NI, 128).transpose(0, 3, 2, 1, 4)
    up_r = up.reshape(E, NH, 128, NI, 128).transpose(0, 3, 2, 1, 4)
    # down slice for (e, h): [128p, NI, 128m] where [p, i, m] = W[i*128+p, h*128+m]
    down_r = down.reshape(E, NI, 128, NH, 128).transpose(0, 3, 2, 1, 4)
    # xT for (e, h): [128p, npad] = X_e^T rows h*128..h*128+127
    xT_r = xbufT.reshape(E, NH, 128, npad)

    nc = _prog_cache.get(npad)
    if nc is None:
        nc = _build_program(npad)
        _prog_cache[npad] = nc

    mm_np = np.float16 if MM_DT == "float16" else np.float32
    in_maps = []
    for c in range(NCORES):
        s = slice(c * EPC, (c + 1) * EPC)
        in_maps.append(
            {
                "xT": np.ascontiguousarray(xT_r[s], dtype=mm_np),
                "gw": np.ascontiguousarray(gate_r[s], dtype=mm_np),
                "uw": np.ascontiguousarray(up_r[s], dtype=mm_np),
                "dw": np.ascontiguousarray(down_r[s], dtype=mm_np),
            }
        )

    trace = bool(os.environ.get("BASS_MOE_TRACE"))
    kwargs = {}
    if trace:
        _install_ntff_hook_shim()
        tcores = os.environ.get("BASS_MOE_TRACE_CORES", "0")
        kwargs = dict(trace=True, trace_cores=[int(c) for c in tcores.split(",")])
    res = run_bass_kernel_spmd(nc, in_maps, core_ids=list(range(NCORES)), **kwargs)
    LAST_EXEC_NS = res.exec_time_ns
    LAST_RESULTS = res

    # ---- host combine ----
    # yT[c] : [EPC, NH, 128, npad] -> O_e [npad, H]
    o_all = np.concatenate([r["yT"] for r in res.results], axis=0)  # [E,NH,128,npad]
    o_all = o_all.reshape(E, H, npad)

    gathered = np.zeros((flat_e.shape[0], H), dtype=np.float32)
    gathered[order[keep]] = o_all[ke, :, kp]
    y = (gathered.reshape(T, K, H) * rw[:, :, None]).sum(axis=1, dtype=np.float32)
    return y.astype(np.float32)


# revision 10
# speedup vs baseline: 1.3970x; 1.0693x over previous
"""MoE experts kernel for Trainium2 (8 NeuronCores, expert-parallel).

Reference computation (per token t, top-k expert e with gate p):
    y[t] = sum_k p[t,k] * down_e @ (silu(x[t] @ gate_e) * (x[t] @ up_e))
with per-expert capacity CAP=1024 (tokens beyond capacity dropped).

Strategy:
  - Host: sort token assignments by expert (stable, matching jnp.argsort),
    build per-expert dense token buffers transposed to [H, NPAD] so the
    device kernel needs no transposes anywhere.
  - Device (SPMD over 8 cores, 8 experts/core): grouped GEMMs in float32r
    (full-rate PE fp32 mode, ~1e-4 rel err):
       G^T = gate^T-slices @ X^T   (accumulate over H chunks)
       U^T = up^T-slices   @ X^T
       Hm^T = silu(G^T) * U^T
       O^T  = down^T-slices @ Hm^T (accumulate over I chunks)
  - Host: gather rows back, apply routing weights, sum over top-k.
"""

import os
import sys

sys.path.insert(0, "/opt/trn_rl_repo")

import numpy as np

E, H, I, T, K = 64, 2048, 768, 4096, 8
CAP = 1024
NCORES = 8
EPC = E // NCORES  # experts per core
NH = H // 128  # 16 contraction chunks for gate/up
NI = I // 128  # 6 contraction chunks for down

_prog_cache = {}
LAST_EXEC_NS = None
LAST_RESULTS = None


def _groups(npad):
    ng = -(-npad // 512)
    w = -(-npad // ng)
    out = []
    s = 0
    while s < npad:
        e = min(s + w, npad)
        out.append((s, e))
        s = e
    return out


MM_DT = "float16"  # matmul operand dtype: float16 | float32r


def _build_program(npad):
    import concourse.bacc as bacc
    import concourse.mybir as mybir
    from concourse.tile import TileContext

    f32 = mybir.dt.float32
    mdt = getattr(mybir.dt, MM_DT)
    SILU = mybir.ActivationFunctionType.Silu

    groups = _groups(npad)

    nc = bacc.Bacc(None, target_bir_lowering=False)
    xT = nc.declare_dram_parameter("xT", [EPC, NH, 128, npad], mdt, isOutput=False)
    gw = nc.declare_dram_parameter("gw", [EPC, NI, 128, NH, 128], mdt, isOutput=False)
    uw = nc.declare_dram_parameter("uw", [EPC, NI, 128, NH, 128], mdt, isOutput=False)
    dw = nc.declare_dram_parameter("dw", [EPC, NH, 128, NI, 128], mdt, isOutput=False)
    yT = nc.declare_dram_parameter("yT", [EPC, NH, 128, npad], f32, isOutput=True)

    xt_bufs = 2

    with TileContext(nc) as tc:
        with (
            tc.sbuf_pool(name="xp", bufs=xt_bufs) as xp,
            tc.sbuf_pool(name="wp", bufs=3) as wp,
            tc.sbuf_pool(name="hp", bufs=2) as hp,
            tc.sbuf_pool(name="op", bufs=3) as op,
            tc.sbuf_pool(name="tp", bufs=3) as tp,
            tc.psum_pool(name="pp", bufs=2) as pp,
        ):
            for e in range(EPC):
                xt_t = xp.tile([128, NH, npad], mdt, name="xt", tag="xt")
                nc.sync.dma_start(out=xt_t, in_=xT[e].rearrange("h p n -> p h n"))
                xts = [xt_t[:, h, :] for h in range(NH)]
                hms = [
                    hp.tile([128, npad], mdt, name=f"hm{i}", tag=f"hm{i}")
                    for i in range(NI)
                ]
                for i in range(NI):
                    g_w = wp.tile([128, NH, 128], mdt, name="g_w", tag="g_w")
                    u_w = wp.tile([128, NH, 128], mdt, name="u_w", tag="u_w")
                    nc.sync.dma_start(out=g_w, in_=gw[e, i, :, :, :])
                    nc.sync.dma_start(out=u_w, in_=uw[e, i, :, :, :])
                    for g0, g1 in groups:
                        wdt = g1 - g0
                        psg = pp.tile([128, wdt], f32, name="psg", tag="psg")
                        psu = pp.tile([128, wdt], f32, name="psu", tag="psu")
                        for h in range(NH):
                            nc.tensor.matmul(
                                psg,
                                g_w[:, h, :],
                                xts[h][:, g0:g1],
                                start=(h == 0),
                                stop=(h == NH - 1),
                            )
                        for h in range(NH):
                            nc.tensor.matmul(
                                psu,
                                u_w[:, h, :],
                                xts[h][:, g0:g1],
                                start=(h == 0),
                                stop=(h == NH - 1),
                            )
                        sil = tp.tile([128, wdt], f32, name="sil", tag="sil")
                        nc.scalar.activation(sil, psg, SILU)
                        nc.vector.tensor_mul(hms[i][:, g0:g1], sil, psu)
                d_w = wp.tile([128, NH, NI, 128], mdt, name="d_w", tag="d_w", bufs=2)
                nc.sync.dma_start(out=d_w, in_=dw[e].rearrange("h p i m -> p h i m"))
                for h in range(NH):
                    ot = op.tile([128, npad], f32, name="ot", tag="ot")
                    for gi, (g0, g1) in enumerate(groups):
                        wdt = g1 - g0
                        pso = pp.tile([128, wdt], f32, name="pso", tag="pso")
                        for i in range(NI):
                            nc.tensor.matmul(
                                pso,
                                d_w[:, h, i, :],
                                hms[i][:, g0:g1],
                                start=(i == 0),
                                stop=(i == NI - 1),
                            )
                        if (h + gi) % 2 == 0:
                            nc.vector.tensor_copy(ot[:, g0:g1], pso)
                        else:
                            nc.scalar.activation(
                                ot[:, g0:g1], pso, mybir.ActivationFunctionType.Copy
                            )
                    nc.gpsimd.dma_start(out=yT[e, h, :, :], in_=ot)
    nc.compile()
    return nc


def _install_neff_cache():
    """Cache walrus NEFF compiles on disk keyed by BIR hash (compile of the
    ~11k-instruction program takes minutes; the BIR is deterministic)."""
    import hashlib
    import shutil

    import concourse.bass2jax as bass2jax
    from concourse.bass_utils import compile_bir_kernel as _orig

    if getattr(bass2jax.compile_bir_kernel, "_moe_cached", False):
        return
    cache_dir = os.environ.get("BASS_NEFF_CACHE", "/tmp/bass_neff_cache")
    os.makedirs(cache_dir, exist_ok=True)

    def cached(bir_json, tmpdir, neff_name="file.neff"):
        key = hashlib.sha256(bir_json).hexdigest()[:24]
        cpath = os.path.join(cache_dir, key + ".neff")
        dst = os.path.join(tmpdir, neff_name)
        if os.path.exists(cpath):
            shutil.copy(cpath, dst)
            return dst
        out = _orig(bir_json, tmpdir, neff_name)
        try:
            shutil.copy(out, cpath)
        except OSError:
            pass
        return out

    cached._moe_cached = True
    bass2jax.compile_bir_kernel = cached


def _install_ntff_hook_shim():
    """Provide antenv.axon_hooks (absent in this container) so that
    run_bass_kernel_spmd(trace=True) can capture NTFF profiles via the
    axon .so — mirrors trn_agent_boot.trn_boot's ctypes hook."""
    import types
    import ctypes
    import contextlib

    if "antenv.axon_hooks" in sys.modules:
        return
    so_path = "/opt/axon/libaxon_pjrt.so"
    lib = ctypes.CDLL(so_path)
    if not hasattr(lib, "axon_start_nrt_profile"):
        return
    lib.axon_start_nrt_profile.argtypes = [
        ctypes.POINTER(ctypes.c_int64),
        ctypes.c_size_t,
    ]
    lib.axon_start_nrt_profile.restype = ctypes.c_int64
    lib.axon_stop_nrt_profile.argtypes = [ctypes.c_char_p]
    lib.axon_stop_nrt_profile.restype = ctypes.c_int64

    @contextlib.contextmanager
    def _hook(output_dir, device_ids):
        import jax

        jax.devices()
        if device_ids:
            ids = (ctypes.c_int64 * len(device_ids))(*device_ids)
            rc = lib.axon_start_nrt_profile(ids, len(device_ids))
        else:
            rc = lib.axon_start_nrt_profile(None, 0)
        if rc != 0:
            raise RuntimeError(f"axon_start_nrt_profile rc={rc}")
        try:
            yield
        finally:
            n = lib.axon_stop_nrt_profile(str(output_dir).encode())
            print(f"profile: {n} file(s) written to {output_dir}", file=sys.stderr)

    mod = types.ModuleType("antenv.axon_hooks")
    mod.get_axon_ntff_profile_hook = lambda: _hook
    mod.set_axon_ntff_profile_hook = lambda h: None
    sys.modules["antenv.axon_hooks"] = mod


def kernel(
    hidden_states,
    routing_weights,
    selected_experts,
    gate_proj,
    up_proj,
    down_proj,
):
    global LAST_EXEC_NS, LAST_RESULTS
    from concourse.bass_utils import run_bass_kernel_spmd

    _install_neff_cache()

    x = np.ascontiguousarray(np.asarray(hidden_states, dtype=np.float32))
    rw = np.asarray(routing_weights, dtype=np.float32)
    sel = np.asarray(selected_experts).astype(np.int64)
    gate = np.asarray(gate_proj, dtype=np.float32)
    up = np.asarray(up_proj, dtype=np.float32)
    down = np.asarray(down_proj, dtype=np.float32)

    # ---- host dispatch (mirrors reference's stable sort-by-expert) ----
    flat_e = sel.reshape(-1)
    order = np.argsort(flat_e, kind="stable")
    sorted_e = flat_e[order]
    counts = np.bincount(flat_e, minlength=E)
    offsets = np.concatenate([[0], np.cumsum(counts)[:-1]])
    pos = np.arange(flat_e.shape[0], dtype=np.int64) - offsets[sorted_e]

    npad = int(min(CAP, max(256, -(-int(counts.max()) // 64) * 64)))
    keep = pos < npad  # npad >= min(max_count, CAP); beyond-CAP tokens dropped

    tok = order // K
    ke = sorted_e[keep]
    kp = pos[keep]

    # Dense per-expert buffers, transposed: xbufT[e] = X_e^T  [H, npad]
    xbufT = np.zeros((E, H, npad), dtype=np.float32)
    xbufT[ke, :, kp] = x[tok[keep]]

    # ---- weight/token layouts (contiguous per-DMA blocks) ----
    # gate/up slice for (e, i): [128p, NH, 128c] where [p, h, c] = W[h*128+p, i*128+c]
    gate_r = gate.reshape(E, NH, 128, NI, 128).transpose(0, 3, 2, 1, 4)
    up_r = up.reshape(E, NH, 128, NI, 128).transpose(0, 3, 2, 1, 4)
    # down slice for (e, h): [128p, NI, 128m] where [p, i, m] = W[i*128+p, h*128+m]
    down_r = down.reshape(E, NI, 128, NH, 128).transpose(0, 3, 2, 1, 4)
    # xT for (e, h): [128p, npad] = X_e^T rows h*128..h*128+127
    xT_r = xbufT.reshape(E, NH, 128, npad)

    nc = _prog_cache.get(npad)
    if nc is None:
        nc = _build_program(npad)
        _prog_cache[npad] = nc

    mm_np = np.float16 if MM_DT == "float16" else np.float32
    in_maps = []
    for c in range(NCORES):
        s = slice(c * EPC, (c + 1) * EPC)
        in_maps.append(
            {
                "xT": np.ascontiguousarray(xT_r[s], dtype=mm_np),
                "gw": np.ascontiguousarray(gate_r[s], dtype=mm_np),
                "uw": np.ascontiguousarray(up_r[s], dtype=mm_np),
                "dw": np.ascontiguousarray(down_r[s], dtype=mm_np),
            }
        )

    trace = bool(os.environ.get("BASS_MOE_TRACE"))
    kwargs = {}
    if trace:
        _install_ntff_hook_shim()
        tcores = os.environ.get("BASS_MOE_TRACE_CORES", "0")
        kwargs = dict(trace=True, trace_cores=[int(c) for c in tcores.split(",")])
    res = run_bass_kernel_spmd(nc, in_maps, core_ids=list(range(NCORES)), **kwargs)
    LAST_EXEC_NS = res.exec_time_ns
    LAST_RESULTS = res

    # ---- host combine ----
    # yT[c] : [EPC, NH, 128, npad] -> O_e [npad, H]
    o_all = np.concatenate([r["yT"] for r in res.results], axis=0)  # [E,NH,128,npad]
    o_all = o_all.reshape(E, H, npad)

    gathered = np.zeros((flat_e.shape[0], H), dtype=np.float32)
    gathered[order[keep]] = o_all[ke, :, kp]
    y = (gathered.reshape(T, K, H) * rw[:, :, None]).sum(axis=1, dtype=np.float32)
    return y.astype(np.float32)


# revision 11
# speedup vs baseline: 1.4218x; 1.0177x over previous
"""MoE experts kernel for Trainium2 (8 NeuronCores, expert-parallel).

Reference computation (per token t, top-k expert e with gate p):
    y[t] = sum_k p[t,k] * down_e @ (silu(x[t] @ gate_e) * (x[t] @ up_e))
with per-expert capacity CAP=1024 (tokens beyond capacity dropped).

Strategy:
  - Host: sort token assignments by expert (stable, matching jnp.argsort),
    build per-expert dense token buffers transposed to [H, NPAD] so the
    device kernel needs no transposes anywhere.
  - Device (SPMD over 8 cores, 8 experts/core): grouped GEMMs in float32r
    (full-rate PE fp32 mode, ~1e-4 rel err):
       G^T = gate^T-slices @ X^T   (accumulate over H chunks)
       U^T = up^T-slices   @ X^T
       Hm^T = silu(G^T) * U^T
       O^T  = down^T-slices @ Hm^T (accumulate over I chunks)
  - Host: gather rows back, apply routing weights, sum over top-k.
"""

import os
import sys

sys.path.insert(0, "/opt/trn_rl_repo")

import numpy as np

E, H, I, T, K = 64, 2048, 768, 4096, 8
CAP = 1024
NCORES = 8
EPC = E // NCORES  # experts per core
NH = H // 128  # 16 contraction chunks for gate/up
NI = I // 128  # 6 contraction chunks for down

_prog_cache = {}
LAST_EXEC_NS = None
LAST_RESULTS = None


def _groups(npad):
    ng = -(-npad // 512)
    w = -(-npad // ng)
    out = []
    s = 0
    while s < npad:
        e = min(s + w, npad)
        out.append((s, e))
        s = e
    return out


MM_DT = "float16"  # matmul operand dtype: float16 | float32r


def _build_program(npad):
    import concourse.bacc as bacc
    import concourse.mybir as mybir
    from concourse.tile import TileContext

    f32 = mybir.dt.float32
    mdt = getattr(mybir.dt, MM_DT)
    SILU = mybir.ActivationFunctionType.Silu

    groups = _groups(npad)

    nc = bacc.Bacc(None, target_bir_lowering=False)
    xT = nc.declare_dram_parameter("xT", [EPC, NH, 128, npad], mdt, isOutput=False)
    gw = nc.declare_dram_parameter("gw", [EPC, NI, 128, NH, 128], mdt, isOutput=False)
    uw = nc.declare_dram_parameter("uw", [EPC, NI, 128, NH, 128], mdt, isOutput=False)
    dw = nc.declare_dram_parameter("dw", [EPC, NH, 128, NI, 128], mdt, isOutput=False)
    yT = nc.declare_dram_parameter("yT", [EPC, NH, 128, npad], f32, isOutput=True)

    xt_bufs = 2

    with TileContext(nc) as tc:
        with (
            tc.sbuf_pool(name="xp", bufs=xt_bufs) as xp,
            tc.sbuf_pool(name="wp", bufs=3) as wp,
            tc.sbuf_pool(name="hp", bufs=2) as hp,
            tc.sbuf_pool(name="op", bufs=3) as op,
            tc.sbuf_pool(name="tp", bufs=3) as tp,
            tc.psum_pool(name="pp", bufs=2) as pp,
        ):
            for e in range(EPC):
                xts = []
                xr = xT[e].rearrange("h p n -> p h n")
                for j in range(4):
                    xt_t = xp.tile([128, 4, npad], mdt, name=f"xt{j}", tag=f"xt{j}")
                    nc.sync.dma_start(out=xt_t, in_=xr[:, 4 * j : 4 * (j + 1), :])
                    xts.extend(xt_t[:, jj, :] for jj in range(4))
                hms = [
                    hp.tile([128, npad], mdt, name=f"hm{i}", tag=f"hm{i}")
                    for i in range(NI)
                ]
                for i in range(NI):
                    g_w = wp.tile([128, NH, 128], mdt, name="g_w", tag="g_w")
                    u_w = wp.tile([128, NH, 128], mdt, name="u_w", tag="u_w")
                    nc.sync.dma_start(out=g_w, in_=gw[e, i, :, :, :])
                    nc.sync.dma_start(out=u_w, in_=uw[e, i, :, :, :])
                    for g0, g1 in groups:
                        wdt = g1 - g0
                        psg = pp.tile([128, wdt], f32, name="psg", tag="psg")
                        psu = pp.tile([128, wdt], f32, name="psu", tag="psu")
                        for h in range(NH):
                            nc.tensor.matmul(
                                psg,
                                g_w[:, h, :],
                                xts[h][:, g0:g1],
                                start=(h == 0),
                                stop=(h == NH - 1),
                            )
                        for h in range(NH):
                            nc.tensor.matmul(
                                psu,
                                u_w[:, h, :],
                                xts[h][:, g0:g1],
                                start=(h == 0),
                                stop=(h == NH - 1),
                            )
                        sil = tp.tile([128, wdt], f32, name="sil", tag="sil")
                        nc.scalar.activation(sil, psg, SILU)
                        nc.vector.tensor_mul(hms[i][:, g0:g1], sil, psu)
                d_w = wp.tile([128, NH, NI, 128], mdt, name="d_w", tag="d_w", bufs=2)
                nc.sync.dma_start(out=d_w, in_=dw[e].rearrange("h p i m -> p h i m"))
                for h in range(NH):
                    ot = op.tile([128, npad], f32, name="ot", tag="ot")
                    for gi, (g0, g1) in enumerate(groups):
                        wdt = g1 - g0
                        pso = pp.tile([128, wdt], f32, name="pso", tag="pso")
                        for i in range(NI):
                            nc.tensor.matmul(
                                pso,
                                d_w[:, h, i, :],
                                hms[i][:, g0:g1],
                                start=(i == 0),
                                stop=(i == NI - 1),
                            )
                        if (h + gi) % 2 == 0:
                            nc.vector.tensor_copy(ot[:, g0:g1], pso)
                        else:
                            nc.scalar.activation(
                                ot[:, g0:g1], pso, mybir.ActivationFunctionType.Copy
                            )
                    eng = nc.gpsimd if h % 2 == 0 else nc.sync
                    eng.dma_start(out=yT[e, h, :, :], in_=ot)
    nc.compile()
    return nc


def _install_neff_cache():
    """Cache walrus NEFF compiles on disk keyed by BIR hash (compile of the
    ~11k-instruction program takes minutes; the BIR is deterministic)."""
    import hashlib
    import shutil

    import concourse.bass2jax as bass2jax
    from concourse.bass_utils import compile_bir_kernel as _orig

    if getattr(bass2jax.compile_bir_kernel, "_moe_cached", False):
        return
    cache_dir = os.environ.get("BASS_NEFF_CACHE", "/tmp/bass_neff_cache")
    os.makedirs(cache_dir, exist_ok=True)

    def cached(bir_json, tmpdir, neff_name="file.neff"):
        key = hashlib.sha256(bir_json).hexdigest()[:24]
        cpath = os.path.join(cache_dir, key + ".neff")
        dst = os.path.join(tmpdir, neff_name)
        if os.path.exists(cpath):
            shutil.copy(cpath, dst)
            return dst
        out = _orig(bir_json, tmpdir, neff_name)
        try:
            shutil.copy(out, cpath)
        except OSError:
            pass
        return out

    cached._moe_cached = True
    bass2jax.compile_bir_kernel = cached


def _install_ntff_hook_shim():
    """Provide antenv.axon_hooks (absent in this container) so that
    run_bass_kernel_spmd(trace=True) can capture NTFF profiles via the
    axon .so — mirrors trn_agent_boot.trn_boot's ctypes hook."""
    import types
    import ctypes
    import contextlib

    if "antenv.axon_hooks" in sys.modules:
        return
    so_path = "/opt/axon/libaxon_pjrt.so"
    lib = ctypes.CDLL(so_path)
    if not hasattr(lib, "axon_start_nrt_profile"):
        return
    lib.axon_start_nrt_profile.argtypes = [
        ctypes.POINTER(ctypes.c_int64),
        ctypes.c_size_t,
    ]
    lib.axon_start_nrt_profile.restype = ctypes.c_int64
    lib.axon_stop_nrt_profile.argtypes = [ctypes.c_char_p]
    lib.axon_stop_nrt_profile.restype = ctypes.c_int64

    @contextlib.contextmanager
    def _hook(output_dir, device_ids):
        import jax

        jax.devices()
        if device_ids:
            ids = (ctypes.c_int64 * len(device_ids))(*device_ids)
            rc = lib.axon_start_nrt_profile(ids, len(device_ids))
        else:
            rc = lib.axon_start_nrt_profile(None, 0)
        if rc != 0:
            raise RuntimeError(f"axon_start_nrt_profile rc={rc}")
        try:
            yield
        finally:
            n = lib.axon_stop_nrt_profile(str(output_dir).encode())
            print(f"profile: {n} file(s) written to {output_dir}", file=sys.stderr)

    mod = types.ModuleType("antenv.axon_hooks")
    mod.get_axon_ntff_profile_hook = lambda: _hook
    mod.set_axon_ntff_profile_hook = lambda h: None
    sys.modules["antenv.axon_hooks"] = mod


def kernel(
    hidden_states,
    routing_weights,
    selected_experts,
    gate_proj,
    up_proj,
    down_proj,
):
    global LAST_EXEC_NS, LAST_RESULTS
    from concourse.bass_utils import run_bass_kernel_spmd

    _install_neff_cache()

    x = np.ascontiguousarray(np.asarray(hidden_states, dtype=np.float32))
    rw = np.asarray(routing_weights, dtype=np.float32)
    sel = np.asarray(selected_experts).astype(np.int64)
    gate = np.asarray(gate_proj, dtype=np.float32)
    up = np.asarray(up_proj, dtype=np.float32)
    down = np.asarray(down_proj, dtype=np.float32)

    # ---- host dispatch (mirrors reference's stable sort-by-expert) ----
    flat_e = sel.reshape(-1)
    order = np.argsort(flat_e, kind="stable")
    sorted_e = flat_e[order]
    counts = np.bincount(flat_e, minlength=E)
    offsets = np.concatenate([[0], np.cumsum(counts)[:-1]])
    pos = np.arange(flat_e.shape[0], dtype=np.int64) - offsets[sorted_e]

    npad = int(min(CAP, max(256, -(-int(counts.max()) // 64) * 64)))
    keep = pos < npad  # npad >= min(max_count, CAP); beyond-CAP tokens dropped

    tok = order // K
    ke = sorted_e[keep]
    kp = pos[keep]

    # Dense per-expert buffers, transposed: xbufT[e] = X_e^T  [H, npad]
    xbufT = np.zeros((E, H, npad), dtype=np.float32)
    xbufT[ke, :, kp] = x[tok[keep]]

    # ---- weight/token layouts (contiguous per-DMA blocks) ----
    # gate/up slice for (e, i): [128p, NH, 128c] where [p, h, c] = W[h*128+p, i*128+c]
    gate_r = gate.reshape(E, NH, 128, NI, 128).transpose(0, 3, 2, 1, 4)
    up_r = up.reshape(E, NH, 128, NI, 128).transpose(0, 3, 2, 1, 4)
    # down slice for (e, h): [128p, NI, 128m] where [p, i, m] = W[i*128+p, h*128+m]
    down_r = down.reshape(E, NI, 128, NH, 128).transpose(0, 3, 2, 1, 4)
    # xT for (e, h): [128p, npad] = X_e^T rows h*128..h*128+127
    xT_r = xbufT.reshape(E, NH, 128, npad)

    nc = _prog_cache.get(npad)
    if nc is None:
        nc = _build_program(npad)
        _prog_cache[npad] = nc

    mm_np = np.float16 if MM_DT == "float16" else np.float32
    in_maps = []
    for c in range(NCORES):
        s = slice(c * EPC, (c + 1) * EPC)
        in_maps.append(
            {
                "xT": np.ascontiguousarray(xT_r[s], dtype=mm_np),
                "gw": np.ascontiguousarray(gate_r[s], dtype=mm_np),
                "uw": np.ascontiguousarray(up_r[s], dtype=mm_np),
                "dw": np.ascontiguousarray(down_r[s], dtype=mm_np),
            }
        )

    trace = bool(os.environ.get("BASS_MOE_TRACE"))
    kwargs = {}
    if trace:
        _install_ntff_hook_shim()
        tcores = os.environ.get("BASS_MOE_TRACE_CORES", "0")
        kwargs = dict(trace=True, trace_cores=[int(c) for c in tcores.split(",")])
    res = run_bass_kernel_spmd(nc, in_maps, core_ids=list(range(NCORES)), **kwargs)
    LAST_EXEC_NS = res.exec_time_ns
    LAST_RESULTS = res

    # ---- host combine ----
    # yT[c] : [EPC, NH, 128, npad] -> O_e [npad, H]
    o_all = np.concatenate([r["yT"] for r in res.results], axis=0)  # [E,NH,128,npad]
    o_all = o_all.reshape(E, H, npad)

    gathered = np.zeros((flat_e.shape[0], H), dtype=np.float32)
    gathered[order[keep]] = o_all[ke, :, kp]
    y = (gathered.reshape(T, K, H) * rw[:, :, None]).sum(axis=1, dtype=np.float32)
    return y.astype(np.float32)
